# revision 10
# baseline (speedup 1.0000x reference)
"""AFT simple attention (causal branch) on 8 TRN2 NeuronCores — hybrid.

out = sigmoid(Q) * cumsum_L( (exp(K+mask) / cumsum_L(exp(K+mask))) * V )

Sharding: data-parallel over (batch n, head-block). Core c handles
n = c // 2 and heads [8*(c%2), 8*(c%2)+8).  Per-core shard = [L=8192,
8*64=512] slab; no cross-core communication.

Within a core the 512 (h,e) rows are split across two pipelines so the
scan work is shared between the TensorEngine and the DVE (under the
8-core power cap the PE runs at ~1.2 GHz, ~0.83 ns/col; the DVE scan
runs at ~2 cyc/elem — neither alone beats ~180us):

  PE side (heads 0..5, 384 cols, quadified layout: L on partitions):
    cumsums as triangular matmuls (U scans + IND colsums + SU/E7 chunk
    carries + SEL carry injection), exactly the original kernel.
  DVE side (heads 6..7, 128 rows, transposed layout: L on free dim):
    cumsums as DVE tensor_tensor_scan chained across 1024-col chunks.

Elementwise work (exp/tanh on ACT, tkv/divide/final STT on DVE) is
shared; the DVE-side chunk lb=c is emitted inside PE-side chunk c so
both instruction streams interleave.  I/O f16; V pre-scaled by 0.5 so
sigmoid folds into (tanh+1); Kw-normalization via a 1-Newton-step
approximate-divide DVE op (~0.2% rel err).
"""

from contextlib import ExitStack

import numpy as np

import concourse.bass as bass
import concourse.tile as tile
from concourse import bacc, mybir
from concourse.bass_utils import run_bass_kernel_spmd

# ---------------- custom DVE op: fused approximate divide ----------------
from concourse import dve_ops as _DO
from concourse.dve_spec import AluOp as _AluOp, Bin as _Bin, Spec as _Spec
from concourse.dve_spec import C0 as _C0, C1 as _C1, Src0 as _Src0, Src1 as _Src1
from concourse.dve_uop import DveOpSpec as _DveOpSpec

_DIV_NAME = "DIVIDE_APPROX_NR1_ANT"
# out = Src0 * y1;  y1 = y0*(C1 - Src1*y0);  y0 = bitcast(~Src1)*C0
_nx = _Bin(_AluOp.BITWISE_NOT, _Src1, _Src1)
_y0 = _nx * _C0
_y1 = _y0 * (_C1 - _Src1 * _y0)


def _ref_div(in0, in1, s0, s1, imm2):
    nx = (~in1.view(np.int32)).view(np.float32)
    y0 = nx * s0
    y1 = y0 * (s1 - in1 * y0)
    return in0 * y1


_DIV_SPEC = _Spec(body=_Src0 * _y1, reference=_ref_div)
_DIV_CONSTS = {"s0": -0.23549792, "s1": 2.0017324}


def _register_divide_op():
    if _DIV_NAME in _DO._SUB_OPCODE_FOR_NAME:
        for op in _DO.OPS:
            if op.name == _DIV_NAME:
                return op
    row = _DO._CUSTOM_DVE_ROW_BASE + len(_DO.OPS)
    assert row < 0x20
    _DO._SUB_OPCODE_FOR_NAME[_DIV_NAME] = row
    shas = {
        ver: _DveOpSpec(
            name=_DIV_NAME, opcode=row, uops=_DO.lower(_DIV_SPEC, ver=ver),
            rd1_en=True,
        ).sha(ver)
        for ver in ("v3", "v4")
    }
    op = _DO.DveOp(_DIV_NAME, _DIV_SPEC, subdim=False, uops_sha=shas)
    _DO.OPS.append(op)
    _DO.CUSTOM_DVE_SPECS[_DIV_NAME] = _DIV_SPEC
    return op


_DIV_OP = _register_divide_op()

N, L, H, E = 4, 8192, 16, 64
NCORES = 8
HPC = H // 2            # heads per core = 8
HP_PE = 6               # heads on the PE pipeline
FREE = HP_PE * E        # 384 PE-side columns
RD = (HPC - HP_PE) * E  # 128 DVE-side rows (one partition block)
P = 128                 # L positions per PE tile / partition block
NTILES = L // P         # 64
CHUNK = 8               # PE tiles per carry chunk
NCHUNKS = NTILES // CHUNK
LB = 1024               # DVE-side L columns per chunk
NLB = L // LB           # 8

F32 = mybir.dt.float32
F16 = mybir.dt.float16
AF = mybir.ActivationFunctionType
ALU = mybir.AluOpType

_CACHE = {}


def _constants():
    f16 = np.float16
    U = np.triu(np.ones((P, P), dtype=np.float32)).astype(f16)
    IND128 = np.zeros((P, CHUNK * P), dtype=np.float32)
    for j in range(CHUNK):
        for m in range(P):
            if m % CHUNK == j:
                IND128[:, j * P + m] = 1.0
    SU = np.triu(np.ones((CHUNK, CHUNK), dtype=np.float32), k=1)   # exclusive prefix
    SEL = np.kron(np.eye(CHUNK, dtype=np.float32), np.ones((1, P), np.float32))
    E7 = np.zeros((CHUNK, CHUNK), dtype=np.float32)
    E7[CHUNK - 1, :] = 1.0               # out[m] += rhs[7]  (broadcast prev totals)
    return U, IND128.astype(f16), SU.astype(f16), SEL.astype(f16), E7.astype(f16)


def _build():
    nc = bacc.Bacc("TRN2", target_bir_lowering=False, debug=False,
                   num_devices=NCORES)
    NQUADS = NTILES // 4
    QB = 4                    # L-blocks per quad tile
    QF = QB * FREE            # 1536
    NQ = CHUNK // QB          # quads per chunk = 2

    q_d = nc.declare_dram_parameter("queries", [NQUADS, P, QF], F16, isOutput=False)
    k_d = nc.declare_dram_parameter("keys", [NQUADS, P, QF], F16, isOutput=False)
    v_d = nc.declare_dram_parameter("values", [NQUADS, P, QF], F16, isOutput=False)
    u_d = nc.declare_dram_parameter("U", [P, P], F16, isOutput=False)
    ind_d = nc.declare_dram_parameter("IND128", [P, CHUNK * P], F16, isOutput=False)
    su_d = nc.declare_dram_parameter("SU", [CHUNK, CHUNK], F16, isOutput=False)
    sel_d = nc.declare_dram_parameter("SEL", [CHUNK, CHUNK * P], F16, isOutput=False)
    e7_d = nc.declare_dram_parameter("E7", [CHUNK, CHUNK], F16, isOutput=False)
    o_d = nc.declare_dram_parameter("out", [NQUADS, P, QF], F16, isOutput=True)
    # DVE-side (transposed) tensors: [NLB, RD, LB]
    k2_d = nc.declare_dram_parameter("keys2", [NLB, RD, LB], F16, isOutput=False)
    v2_d = nc.declare_dram_parameter("values2", [NLB, RD, LB], F16, isOutput=False)
    q2_d = nc.declare_dram_parameter("queries2", [NLB, RD, LB], F16, isOutput=False)
    o2_d = nc.declare_dram_parameter("out2", [NLB, RD, LB], F16, isOutput=True)

    with ExitStack() as ctx:
        tc = ctx.enter_context(tile.TileContext(nc))
        const = ctx.enter_context(tc.tile_pool(name="const", bufs=1))
        pk = ctx.enter_context(tc.tile_pool(name="k", bufs=3))
        pke = ctx.enter_context(tc.tile_pool(name="ke", bufs=4))
        pv = ctx.enter_context(tc.tile_pool(name="v", bufs=3))
        pt = ctx.enter_context(tc.tile_pool(name="tkv", bufs=3))
        pt2 = ctx.enter_context(tc.tile_pool(name="t2", bufs=6))
        pq = ctx.enter_context(tc.tile_pool(name="qo", bufs=4))
        psmall = ctx.enter_context(tc.tile_pool(name="small", bufs=2))
        ps_big = ctx.enter_context(tc.tile_pool(name="ps_big", bufs=6, space="PSUM"))
        ps_sm = ctx.enter_context(tc.tile_pool(name="ps_sm", bufs=1, space="PSUM"))
        # DVE-side pools
        dk = ctx.enter_context(tc.tile_pool(name="dk", bufs=3))
        dke = ctx.enter_context(tc.tile_pool(name="dke", bufs=3))
        ds1 = ctx.enter_context(tc.tile_pool(name="ds1", bufs=3))
        dv = ctx.enter_context(tc.tile_pool(name="dv", bufs=3))
        dt = ctx.enter_context(tc.tile_pool(name="dt", bufs=3))
        dt2 = ctx.enter_context(tc.tile_pool(name="dt2", bufs=3))
        ds2 = ctx.enter_context(tc.tile_pool(name="ds2", bufs=3))
        dq = ctx.enter_context(tc.tile_pool(name="dq", bufs=3))
        do = ctx.enter_context(tc.tile_pool(name="do", bufs=3))

        U = const.tile([P, P], F16, name="U")
        nc.sync.dma_start(U[:], u_d[:])
        IND = const.tile([P, CHUNK * P], F16, name="IND")
        nc.sync.dma_start(IND[:], ind_d[:])
        SU = const.tile([CHUNK, CHUNK], F16, name="SU")
        nc.sync.dma_start(SU[:], su_d[:])
        SEL = const.tile([CHUNK, CHUNK * P], F16, name="SEL")
        nc.sync.dma_start(SEL[:], sel_d[:])
        E7 = const.tile([CHUNK, CHUNK], F16, name="E7")
        nc.sync.dma_start(E7[:], e7_d[:])

        def quad_load(pool, tag, dram, t0):
            qt = pool.tile([P, QF], F16, tag=tag, name=tag)
            nc.sync.dma_start(qt[:], dram[t0 // QB])
            return qt

        # ---- DVE-side chunk step (lb): transposed layout, scans on DVE
        dstate = {"s1": None, "s2": None}

        def dve_side_step(lb):
            kt = dk.tile([RD, LB], F16, tag="dk", name="dk")
            nc.sync.dma_start(kt[:], k2_d[lb])
            vt = dv.tile([RD, LB], F16, tag="dv", name="dv")
            nc.sync.dma_start(vt[:], v2_d[lb])
            qt = dq.tile([RD, LB], F16, tag="dq", name="dq")
            nc.sync.dma_start(qt[:], q2_d[lb])

            ke = dke.tile([RD, LB], F16, tag="dke", name="dke")
            nc.scalar.activation(ke[:], kt[:], AF.Exp)
            nc.scalar.activation(qt[:], qt[:], AF.Tanh, scale=0.5)

            s1 = ds1.tile([RD, LB], F32, tag="ds1", name="ds1")
            init1 = 0.0 if lb == 0 else dstate["s1"][:, LB - 1:LB]
            nc.vector.tensor_tensor_scan(
                s1[:], ke[:], ke[:], init1, ALU.add, ALU.bypass)
            dstate["s1"] = s1

            tkv = dt.tile([RD, LB], F16, tag="dt", name="dt")
            nc.vector.tensor_tensor(tkv[:], ke[:], vt[:], ALU.mult)

            t2 = dt2.tile([RD, LB], F16, tag="dt2", name="dt2")
            nc.vector._custom_dve(
                _DIV_OP, out=t2[:], in0=tkv[:], in1=s1[:],
                s0=_DIV_CONSTS["s0"], s1=_DIV_CONSTS["s1"], imm2=0.0)

            s2 = ds2.tile([RD, LB], F16, tag="ds2", name="ds2")
            init2 = 0.0 if lb == 0 else dstate["s2"][:, LB - 1:LB]
            nc.vector.tensor_tensor_scan(
                s2[:], t2[:], t2[:], init2, ALU.add, ALU.bypass)
            dstate["s2"] = s2

            ot = do.tile([RD, LB], F16, tag="do", name="do")
            nc.vector.scalar_tensor_tensor(
                ot[:], qt[:], 1.0, s2[:], ALU.add, ALU.mult)
            nc.scalar.dma_start(o2_d[lb], ot[:])

        prev1 = None      # (scol1_sb, c1_sb) of previous chunk
        prev2 = None
        pend_e = None

        def phase_a(c):
            """Load K quads of chunk c, exp, colsum matmuls, V loads + tkv.
            Returns (keq, tkv, t2q, scol1)."""
            t0 = c * CHUNK
            keq = []
            scol1 = ps_sm.tile([P, FREE], F32, tag="scol1")
            for u in range(NQ):
                kq = quad_load(pk, "k", k_d, t0 + u * QB)
                ke = pke.tile([P, QF], F16, tag="ke", name="ke")
                keq.append(ke)
                nc.scalar.activation(ke[:], kq[:], AF.Exp)
                for b in range(QB):
                    j = u * QB + b
                    sl = slice(b * FREE, (b + 1) * FREE)
                    nc.tensor.matmul(
                        scol1[:], IND[:, j * P:(j + 1) * P], ke[:, sl],
                        start=(j == 0), stop=(j == CHUNK - 1),
                    )
            tkv = []
            t2q = []
            for u in range(NQ):
                vq = quad_load(pv, "v", v_d, t0 + u * QB)
                tkv.append(pt.tile([P, QF], F16, tag="tkv", name="tkv"))
                t2q.append(pt2.tile([P, QF], F16, tag="t2", name="t2"))
                nc.vector.tensor_tensor(tkv[u][:], keq[u][:], vq[:], ALU.mult)
            return keq, tkv, t2q, scol1

        nxt = phase_a(0)
        for c in range(NCHUNKS):
            t0 = c * CHUNK
            keq, tkv, t2q, scol1 = nxt

            # ---- phase B: chunk carry for scan 1 (base folded in via E7)
            scol1_sb = psmall.tile([CHUNK, FREE], F16, tag="scol1_sb")
            nc.scalar.copy(scol1_sb[:], scol1[0:CHUNK, :])
            c1_ps = ps_sm.tile([CHUNK, FREE], F32, tag="scol1", name="c1_ps")
            nc.tensor.matmul(c1_ps[:], SU[:], scol1_sb[:], start=True,
                             stop=(prev1 is None))
            if prev1 is not None:
                nc.tensor.matmul(c1_ps[:], E7[:], prev1[0][:], start=False, stop=False)
                nc.tensor.matmul(c1_ps[:], E7[:], prev1[1][:], start=False, stop=True)
            c1_sb = psmall.tile([CHUNK, FREE], F16, tag="c1_sb")
            nc.scalar.copy(c1_sb[:], c1_ps[:])
            prev1 = (scol1_sb, c1_sb)

            # ---- emit lagged phase E(c-1): PE filler while carry copies run
            if pend_e is not None:
                pend_e()

            # ---- phase C: scan1 per tile + fused divide; scan2 colsums.
            scol2 = ps_sm.tile([P, FREE], F32, tag="scol2")
            for h in range(2):
                s_list = []
                for j in range(h * 4, h * 4 + 4):
                    u, b = j // QB, j % QB
                    sl = slice(b * FREE, (b + 1) * FREE)
                    s_ps = ps_big.tile([P, FREE], F32, tag="ps_big", name="s_ps")
                    s_list.append((s_ps, u, sl))
                    nc.tensor.matmul(s_ps[:], U[:], keq[u][:, sl], start=True, stop=False)
                for i, j in enumerate(range(h * 4, h * 4 + 4)):
                    nc.tensor.matmul(s_list[i][0][:], SEL[:, j * P:(j + 1) * P],
                                     c1_sb[:], start=False, stop=True)
                for s_ps, u, sl in s_list:
                    nc.vector._custom_dve(
                        _DIV_OP, out=t2q[u][:, sl], in0=tkv[u][:, sl], in1=s_ps[:],
                        s0=_DIV_CONSTS["s0"], s1=_DIV_CONSTS["s1"], imm2=0.0)
                for j in range(h * 4, h * 4 + 4):
                    u, b = j // QB, j % QB
                    sl = slice(b * FREE, (b + 1) * FREE)
                    nc.tensor.matmul(
                        scol2[:], IND[:, j * P:(j + 1) * P], t2q[u][:, sl],
                        start=(j == 0), stop=(j == CHUNK - 1),
                    )
                if h == 0:
                    # interleave the DVE-side chunk mid-PE-chunk: the DVE
                    # scans/divide slot in after this half's divides.
                    dve_side_step(c)

            # ---- hoisted phase A of chunk c+1: PE filler while the D-phase
            # carry copies (ACT) run, so the PE stream has no gap at D.
            if c + 1 < NCHUNKS:
                nxt = phase_a(c + 1)

            # ---- phase D: chunk carry for scan 2
            scol2_sb = psmall.tile([CHUNK, FREE], F16, tag="scol2_sb")
            nc.scalar.copy(scol2_sb[:], scol2[0:CHUNK, :])
            c2_ps = ps_sm.tile([CHUNK, FREE], F32, tag="scol2", name="c2_ps")
            nc.tensor.matmul(c2_ps[:], SU[:], scol2_sb[:], start=True,
                             stop=(prev2 is None))
            if prev2 is not None:
                nc.tensor.matmul(c2_ps[:], E7[:], prev2[0][:], start=False, stop=False)
                nc.tensor.matmul(c2_ps[:], E7[:], prev2[1][:], start=False, stop=True)
            c2_sb = psmall.tile([CHUNK, FREE], F16, tag="c2_sb")
            nc.scalar.copy(c2_sb[:], c2_ps[:])
            prev2 = (scol2_sb, c2_sb)

            # ---- phase E (emitted with a 1-chunk lag)
            def phase_e(t0=t0, t2q=t2q, c2_sb=c2_sb):
                for u in range(NQ):
                    qq = quad_load(pq, "qo", q_d, t0 + u * QB)
                    nc.scalar.activation(qq[:], qq[:], AF.Tanh, scale=0.5)
                    w_list = []
                    for b in range(QB):
                        j = u * QB + b
                        sl = slice(b * FREE, (b + 1) * FREE)
                        w_ps = ps_big.tile([P, FREE], F32, tag="ps_big", name="w_ps")
                        w_list.append((w_ps, sl))
                        nc.tensor.matmul(w_ps[:], U[:], t2q[u][:, sl], start=True, stop=False)
                    for b in range(QB):
                        j = u * QB + b
                        nc.tensor.matmul(w_list[b][0][:], SEL[:, j * P:(j + 1) * P],
                                         c2_sb[:], start=False, stop=True)
                    for w_ps, sl in w_list:
                        # out = (tanh(q/2) + 1) * s2' = sigmoid(q) * s2
                        nc.vector.scalar_tensor_tensor(
                            qq[:, sl], qq[:, sl], 1.0, w_ps[:], ALU.add, ALU.mult)
                    nc.scalar.dma_start(o_d[(t0 + u * QB) // QB], qq[:])
            pend_e = phase_e
        if pend_e is not None:
            pend_e()
    nc.compile()
    return nc


def _get_nc():
    if "nc" not in _CACHE:
        _CACHE["nc"] = _build()
    return _CACHE["nc"]


def _quadify(a):
    """[L, FREE] -> DMA-native [NTILES//4, P, 4*FREE] staged layout."""
    return np.ascontiguousarray(
        a.reshape(NTILES // 4, 4, P, FREE).transpose(0, 2, 1, 3)
    ).reshape(NTILES // 4, P, 4 * FREE)


def _stage2(a):
    """[L, RD] -> DVE-side [NLB, RD, LB] transposed layout."""
    # [L, RD] -> [RD, L] -> [RD, NLB, LB] -> [NLB, RD, LB]
    return np.ascontiguousarray(a.T.reshape(RD, NLB, LB).transpose(1, 0, 2))


def _run(queries, keys, values, key_lengths_add, trace=False, **kw):
    nc = _get_nc()
    U, IND128, SU, SEL, E7 = _constants()
    in_maps = []
    for c in range(NCORES):
        n = c // 2
        h0 = (c % 2) * HPC
        kk = keys[n, :, h0:h0 + HPC, :].reshape(L, HPC * E) \
            + key_lengths_add[n][:, None]
        qq = queries[n, :, h0:h0 + HPC, :].reshape(L, HPC * E)
        vv = values[n, :, h0:h0 + HPC, :].reshape(L, HPC * E) * 0.5
        kk16 = kk.astype(np.float16)
        qq16 = qq.astype(np.float16)
        vv16 = vv.astype(np.float16)
        in_maps.append({
            "queries": _quadify(qq16[:, :FREE]),
            "keys": _quadify(kk16[:, :FREE]),
            "values": _quadify(vv16[:, :FREE]),
            "queries2": _stage2(qq16[:, FREE:]),
            "keys2": _stage2(kk16[:, FREE:]),
            "values2": _stage2(vv16[:, FREE:]),
            "U": U, "IND128": IND128, "SU": SU, "SEL": SEL, "E7": E7,
        })
    res = run_bass_kernel_spmd(nc, in_maps, core_ids=list(range(NCORES)),
                               trace=trace, **kw)
    out = np.empty((N, L, H, E), dtype=np.float32)
    for c in range(NCORES):
        n = c // 2
        h0 = (c % 2) * HPC
        oc = res.results[c]["out"].reshape(NTILES // 4, P, 4, FREE)
        oc = oc.transpose(0, 2, 1, 3).reshape(L, FREE)
        o2 = res.results[c]["out2"]            # [NLB, RD, LB]
        o2 = o2.transpose(1, 0, 2).reshape(RD, L).T    # [L, RD]
        full = np.concatenate([oc, o2], axis=1)        # [L, 512]
        out[n, :, h0:h0 + HPC, :] = full.astype(np.float32).reshape(L, HPC, E)
    return out, res


def kernel(queries, keys, values, key_lengths_add):
    out, _ = _run(queries, keys, values, key_lengths_add)
    return out


if __name__ == "__main__":
    rng = np.random.default_rng(0)
    q = rng.standard_normal((N, L, H, E), dtype=np.float32)
    k = rng.standard_normal((N, L, H, E), dtype=np.float32)
    v = rng.standard_normal((N, L, H, E), dtype=np.float32)
    m = np.zeros((N, L), dtype=np.float32)
    o = kernel(q, k, v, m)
    print(o.shape, o.dtype, np.abs(o).mean())


# revision 12
# speedup vs baseline: 1.0663x; 1.0663x over previous
"""AFT simple attention (causal branch) on 8 TRN2 NeuronCores — hybrid.

out = sigmoid(Q) * cumsum_L( (exp(K+mask) / cumsum_L(exp(K+mask))) * V )

Sharding: data-parallel over (batch n, head-block). Core c handles
n = c // 2 and heads [8*(c%2), 8*(c%2)+8).  Per-core shard = [L=8192,
8*64=512] slab; no cross-core communication.

Within a core the 512 (h,e) rows are split across two pipelines so the
scan work is shared between the TensorEngine and the DVE (under the
8-core power cap the PE runs at ~1.2 GHz, ~0.83 ns/col; the DVE scan
runs at ~2 cyc/elem — neither alone beats ~180us):

  PE side (heads 0..5, 384 cols, quadified layout: L on partitions):
    cumsums as triangular matmuls (U scans + IND colsums + SU/E7 chunk
    carries + SEL carry injection), exactly the original kernel.
  DVE side (heads 6..7, 128 rows, transposed layout: L on free dim):
    cumsums as DVE tensor_tensor_scan chained across 1024-col chunks.

Elementwise work (exp/tanh on ACT, tkv/divide/final STT on DVE) is
shared; the DVE-side chunk lb=c is emitted inside PE-side chunk c so
both instruction streams interleave.  I/O f16; V pre-scaled by 0.5 so
sigmoid folds into (tanh+1); Kw-normalization via a 1-Newton-step
approximate-divide DVE op (~0.2% rel err).
"""

from contextlib import ExitStack

import numpy as np

import concourse.bass as bass
import concourse.tile as tile
from concourse import bacc, mybir
from concourse.bass_utils import run_bass_kernel_spmd

# ---------------- custom DVE op: fused approximate divide ----------------
from concourse import dve_ops as _DO
from concourse.dve_spec import AluOp as _AluOp, Bin as _Bin, Spec as _Spec
from concourse.dve_spec import C0 as _C0, C1 as _C1, Src0 as _Src0, Src1 as _Src1
from concourse.dve_uop import DveOpSpec as _DveOpSpec

_DIV_NAME = "DIVIDE_APPROX_NR1_ANT"
# out = Src0 * y1;  y1 = y0*(C1 - Src1*y0);  y0 = bitcast(~Src1)*C0
_nx = _Bin(_AluOp.BITWISE_NOT, _Src1, _Src1)
_y0 = _nx * _C0
_y1 = _y0 * (_C1 - _Src1 * _y0)


def _ref_div(in0, in1, s0, s1, imm2):
    nx = (~in1.view(np.int32)).view(np.float32)
    y0 = nx * s0
    y1 = y0 * (s1 - in1 * y0)
    return in0 * y1


_DIV_SPEC = _Spec(body=_Src0 * _y1, reference=_ref_div)
_DIV_CONSTS = {"s0": -0.23549792, "s1": 2.0017324}


def _register_divide_op():
    if _DIV_NAME in _DO._SUB_OPCODE_FOR_NAME:
        for op in _DO.OPS:
            if op.name == _DIV_NAME:
                return op
    row = _DO._CUSTOM_DVE_ROW_BASE + len(_DO.OPS)
    assert row < 0x20
    _DO._SUB_OPCODE_FOR_NAME[_DIV_NAME] = row
    shas = {
        ver: _DveOpSpec(
            name=_DIV_NAME, opcode=row, uops=_DO.lower(_DIV_SPEC, ver=ver),
            rd1_en=True,
        ).sha(ver)
        for ver in ("v3", "v4")
    }
    op = _DO.DveOp(_DIV_NAME, _DIV_SPEC, subdim=False, uops_sha=shas)
    _DO.OPS.append(op)
    _DO.CUSTOM_DVE_SPECS[_DIV_NAME] = _DIV_SPEC
    return op


_DIV_OP = _register_divide_op()

N, L, H, E = 4, 8192, 16, 64
NCORES = 8
HPC = H // 2            # heads per core = 8
HP_PE = 6               # heads on the PE pipeline
FREE = HP_PE * E        # 384 PE-side columns
RD = (HPC - HP_PE) * E  # 128 DVE-side rows (one partition block)
P = 128                 # L positions per PE tile / partition block
NTILES = L // P         # 64
CHUNK = 8               # PE tiles per carry chunk
NCHUNKS = NTILES // CHUNK
LB = 1024               # DVE-side L columns per chunk
NLB = L // LB           # 8

F32 = mybir.dt.float32
F16 = mybir.dt.float16
AF = mybir.ActivationFunctionType
ALU = mybir.AluOpType

_CACHE = {}


def _constants():
    f16 = np.float16
    U = np.triu(np.ones((P, P), dtype=np.float32)).astype(f16)
    IND128 = np.zeros((P, CHUNK * P), dtype=np.float32)
    for j in range(CHUNK):
        for m in range(P):
            if m % CHUNK == j:
                IND128[:, j * P + m] = 1.0
    SU = np.triu(np.ones((CHUNK, CHUNK), dtype=np.float32), k=1)   # exclusive prefix
    SEL = np.kron(np.eye(CHUNK, dtype=np.float32), np.ones((1, P), np.float32))
    E7 = np.zeros((CHUNK, CHUNK), dtype=np.float32)
    E7[CHUNK - 1, :] = 1.0               # out[m] += rhs[7]  (broadcast prev totals)
    return U, IND128.astype(f16), SU.astype(f16), SEL.astype(f16), E7.astype(f16)


def _build():
    nc = bacc.Bacc("TRN2", target_bir_lowering=False, debug=False,
                   num_devices=NCORES)
    NQUADS = NTILES // 4
    QB = 4                    # L-blocks per quad tile
    QF = QB * FREE            # 1536
    NQ = CHUNK // QB          # quads per chunk = 2

    q_d = nc.declare_dram_parameter("queries", [NQUADS, P, QF], F16, isOutput=False)
    k_d = nc.declare_dram_parameter("keys", [NQUADS, P, QF], F16, isOutput=False)
    v_d = nc.declare_dram_parameter("values", [NQUADS, P, QF], F16, isOutput=False)
    u_d = nc.declare_dram_parameter("U", [P, P], F16, isOutput=False)
    ind_d = nc.declare_dram_parameter("IND128", [P, CHUNK * P], F16, isOutput=False)
    su_d = nc.declare_dram_parameter("SU", [CHUNK, CHUNK], F16, isOutput=False)
    sel_d = nc.declare_dram_parameter("SEL", [CHUNK, CHUNK * P], F16, isOutput=False)
    e7_d = nc.declare_dram_parameter("E7", [CHUNK, CHUNK], F16, isOutput=False)
    o_d = nc.declare_dram_parameter("out", [NQUADS, P, QF], F16, isOutput=True)
    # DVE-side (transposed) tensors: [NLB, RD, LB]
    k2_d = nc.declare_dram_parameter("keys2", [NLB, RD, LB], F16, isOutput=False)
    v2_d = nc.declare_dram_parameter("values2", [NLB, RD, LB], F16, isOutput=False)
    q2_d = nc.declare_dram_parameter("queries2", [NLB, RD, LB], F16, isOutput=False)
    o2_d = nc.declare_dram_parameter("out2", [NLB, RD, LB], F16, isOutput=True)

    with ExitStack() as ctx:
        tc = ctx.enter_context(tile.TileContext(nc))
        const = ctx.enter_context(tc.tile_pool(name="const", bufs=1))
        pk = ctx.enter_context(tc.tile_pool(name="k", bufs=3))
        pke = ctx.enter_context(tc.tile_pool(name="ke", bufs=4))
        pv = ctx.enter_context(tc.tile_pool(name="v", bufs=3))
        pt = ctx.enter_context(tc.tile_pool(name="tkv", bufs=3))
        pt2 = ctx.enter_context(tc.tile_pool(name="t2", bufs=6))
        pq = ctx.enter_context(tc.tile_pool(name="qo", bufs=4))
        psmall = ctx.enter_context(tc.tile_pool(name="small", bufs=2))
        ps_big = ctx.enter_context(tc.tile_pool(name="ps_big", bufs=6, space="PSUM"))
        ps_sm = ctx.enter_context(tc.tile_pool(name="ps_sm", bufs=1, space="PSUM"))
        # DVE-side pools
        dk = ctx.enter_context(tc.tile_pool(name="dk", bufs=3))
        dke = ctx.enter_context(tc.tile_pool(name="dke", bufs=3))
        ds1 = ctx.enter_context(tc.tile_pool(name="ds1", bufs=3))
        dv = ctx.enter_context(tc.tile_pool(name="dv", bufs=3))
        dt = ctx.enter_context(tc.tile_pool(name="dt", bufs=3))
        dt2 = ctx.enter_context(tc.tile_pool(name="dt2", bufs=3))
        ds2 = ctx.enter_context(tc.tile_pool(name="ds2", bufs=3))
        dq = ctx.enter_context(tc.tile_pool(name="dq", bufs=3))
        do = ctx.enter_context(tc.tile_pool(name="do", bufs=3))

        U = const.tile([P, P], F16, name="U")
        nc.sync.dma_start(U[:], u_d[:])
        IND = const.tile([P, CHUNK * P], F16, name="IND")
        nc.sync.dma_start(IND[:], ind_d[:])
        SU = const.tile([CHUNK, CHUNK], F16, name="SU")
        nc.sync.dma_start(SU[:], su_d[:])
        SEL = const.tile([CHUNK, CHUNK * P], F16, name="SEL")
        nc.sync.dma_start(SEL[:], sel_d[:])
        E7 = const.tile([CHUNK, CHUNK], F16, name="E7")
        nc.sync.dma_start(E7[:], e7_d[:])

        def quad_load(pool, tag, dram, t0):
            qt = pool.tile([P, QF], F16, tag=tag, name=tag)
            nc.sync.dma_start(qt[:], dram[t0 // QB])
            return qt

        # ---- DVE-side chunk step (lb): transposed layout, scans on DVE.
        # Split in two halves so the DVE-side work interleaves with the
        # PE-side divides in ~6us slices instead of one 12us block.
        dstate = {"s1": None, "s2": None, "t2": None, "qt": None}

        def dve_side_half1(lb):
            kt = dk.tile([RD, LB], F16, tag="dk", name="dk")
            nc.sync.dma_start(kt[:], k2_d[lb])
            vt = dv.tile([RD, LB], F16, tag="dv", name="dv")
            nc.sync.dma_start(vt[:], v2_d[lb])
            qt = dq.tile([RD, LB], F16, tag="dq", name="dq")
            nc.sync.dma_start(qt[:], q2_d[lb])

            ke = dke.tile([RD, LB], F16, tag="dke", name="dke")
            nc.scalar.activation(ke[:], kt[:], AF.Exp)
            nc.scalar.activation(qt[:], qt[:], AF.Tanh, scale=0.5)
            dstate["qt"] = qt

            s1 = ds1.tile([RD, LB], F32, tag="ds1", name="ds1")
            init1 = 0.0 if lb == 0 else dstate["s1"][:, LB - 1:LB]
            nc.vector.tensor_tensor_scan(
                s1[:], ke[:], ke[:], init1, ALU.add, ALU.bypass)
            dstate["s1"] = s1

            tkv = dt.tile([RD, LB], F16, tag="dt", name="dt")
            nc.vector.tensor_tensor(tkv[:], ke[:], vt[:], ALU.mult)

            t2 = dt2.tile([RD, LB], F16, tag="dt2", name="dt2")
            nc.vector._custom_dve(
                _DIV_OP, out=t2[:], in0=tkv[:], in1=s1[:],
                s0=_DIV_CONSTS["s0"], s1=_DIV_CONSTS["s1"], imm2=0.0)
            dstate["t2"] = t2

        def dve_side_half2(lb):
            t2, qt = dstate["t2"], dstate["qt"]
            s2 = ds2.tile([RD, LB], F16, tag="ds2", name="ds2")
            init2 = 0.0 if lb == 0 else dstate["s2"][:, LB - 1:LB]
            nc.vector.tensor_tensor_scan(
                s2[:], t2[:], t2[:], init2, ALU.add, ALU.bypass)
            dstate["s2"] = s2

            ot = do.tile([RD, LB], F16, tag="do", name="do")
            nc.vector.scalar_tensor_tensor(
                ot[:], qt[:], 1.0, s2[:], ALU.add, ALU.mult)
            nc.scalar.dma_start(o2_d[lb], ot[:])

        prev1 = None      # (scol1_sb, c1_sb) of previous chunk
        prev2 = None
        pend_e = None

        def phase_a(c):
            """Load K quads of chunk c, exp, colsum matmuls, V loads + tkv.
            Returns (keq, tkv, t2q, scol1)."""
            t0 = c * CHUNK
            keq = []
            scol1 = ps_sm.tile([P, FREE], F32, tag="scol1")
            for u in range(NQ):
                kq = quad_load(pk, "k", k_d, t0 + u * QB)
                ke = pke.tile([P, QF], F16, tag="ke", name="ke")
                keq.append(ke)
                nc.scalar.activation(ke[:], kq[:], AF.Exp)
                for b in range(QB):
                    j = u * QB + b
                    sl = slice(b * FREE, (b + 1) * FREE)
                    nc.tensor.matmul(
                        scol1[:], IND[:, j * P:(j + 1) * P], ke[:, sl],
                        start=(j == 0), stop=(j == CHUNK - 1),
                    )
            tkv = []
            t2q = []
            for u in range(NQ):
                vq = quad_load(pv, "v", v_d, t0 + u * QB)
                tkv.append(pt.tile([P, QF], F16, tag="tkv", name="tkv"))
                t2q.append(pt2.tile([P, QF], F16, tag="t2", name="t2"))
                nc.vector.tensor_tensor(tkv[u][:], keq[u][:], vq[:], ALU.mult)
            return keq, tkv, t2q, scol1

        nxt = phase_a(0)
        for c in range(NCHUNKS):
            t0 = c * CHUNK
            keq, tkv, t2q, scol1 = nxt

            # ---- phase B: chunk carry for scan 1 (base folded in via E7)
            scol1_sb = psmall.tile([CHUNK, FREE], F16, tag="scol1_sb")
            nc.scalar.copy(scol1_sb[:], scol1[0:CHUNK, :])
            c1_ps = ps_sm.tile([CHUNK, FREE], F32, tag="scol1", name="c1_ps")
            nc.tensor.matmul(c1_ps[:], SU[:], scol1_sb[:], start=True,
                             stop=(prev1 is None))
            if prev1 is not None:
                nc.tensor.matmul(c1_ps[:], E7[:], prev1[0][:], start=False, stop=False)
                nc.tensor.matmul(c1_ps[:], E7[:], prev1[1][:], start=False, stop=True)
            c1_sb = psmall.tile([CHUNK, FREE], F16, tag="c1_sb")
            nc.scalar.copy(c1_sb[:], c1_ps[:])
            prev1 = (scol1_sb, c1_sb)

            # ---- emit lagged phase E(c-1): PE filler while carry copies run
            if pend_e is not None:
                pend_e()

            # ---- phase C: scan1 per tile + fused divide; scan2 colsums.
            scol2 = ps_sm.tile([P, FREE], F32, tag="scol2")
            for h in range(2):
                s_list = []
                for j in range(h * 4, h * 4 + 4):
                    u, b = j // QB, j % QB
                    sl = slice(b * FREE, (b + 1) * FREE)
                    s_ps = ps_big.tile([P, FREE], F32, tag="ps_big", name="s_ps")
                    s_list.append((s_ps, u, sl))
                    nc.tensor.matmul(s_ps[:], U[:], keq[u][:, sl], start=True, stop=False)
                for i, j in enumerate(range(h * 4, h * 4 + 4)):
                    nc.tensor.matmul(s_list[i][0][:], SEL[:, j * P:(j + 1) * P],
                                     c1_sb[:], start=False, stop=True)
                for s_ps, u, sl in s_list:
                    nc.vector._custom_dve(
                        _DIV_OP, out=t2q[u][:, sl], in0=tkv[u][:, sl], in1=s_ps[:],
                        s0=_DIV_CONSTS["s0"], s1=_DIV_CONSTS["s1"], imm2=0.0)
                for j in range(h * 4, h * 4 + 4):
                    u, b = j // QB, j % QB
                    sl = slice(b * FREE, (b + 1) * FREE)
                    nc.tensor.matmul(
                        scol2[:], IND[:, j * P:(j + 1) * P], t2q[u][:, sl],
                        start=(j == 0), stop=(j == CHUNK - 1),
                    )
                if h == 0:
                    dve_side_half1(c)
                else:
                    dve_side_half2(c)

            # ---- hoisted phase A of chunk c+1: PE filler while the D-phase
            # carry copies (ACT) run, so the PE stream has no gap at D.
            if c + 1 < NCHUNKS:
                nxt = phase_a(c + 1)

            # ---- phase D: chunk carry for scan 2
            scol2_sb = psmall.tile([CHUNK, FREE], F16, tag="scol2_sb")
            nc.scalar.copy(scol2_sb[:], scol2[0:CHUNK, :])
            c2_ps = ps_sm.tile([CHUNK, FREE], F32, tag="scol2", name="c2_ps")
            nc.tensor.matmul(c2_ps[:], SU[:], scol2_sb[:], start=True,
                             stop=(prev2 is None))
            if prev2 is not None:
                nc.tensor.matmul(c2_ps[:], E7[:], prev2[0][:], start=False, stop=False)
                nc.tensor.matmul(c2_ps[:], E7[:], prev2[1][:], start=False, stop=True)
            c2_sb = psmall.tile([CHUNK, FREE], F16, tag="c2_sb")
            nc.scalar.copy(c2_sb[:], c2_ps[:])
            prev2 = (scol2_sb, c2_sb)

            # ---- phase E (emitted with a 1-chunk lag)
            def phase_e(t0=t0, t2q=t2q, c2_sb=c2_sb):
                for u in range(NQ):
                    qq = quad_load(pq, "qo", q_d, t0 + u * QB)
                    nc.scalar.activation(qq[:], qq[:], AF.Tanh, scale=0.5)
                    w_list = []
                    for b in range(QB):
                        j = u * QB + b
                        sl = slice(b * FREE, (b + 1) * FREE)
                        w_ps = ps_big.tile([P, FREE], F32, tag="ps_big", name="w_ps")
                        w_list.append((w_ps, sl))
                        nc.tensor.matmul(w_ps[:], U[:], t2q[u][:, sl], start=True, stop=False)
                    for b in range(QB):
                        j = u * QB + b
                        nc.tensor.matmul(w_list[b][0][:], SEL[:, j * P:(j + 1) * P],
                                         c2_sb[:], start=False, stop=True)
                    for w_ps, sl in w_list:
                        # out = (tanh(q/2) + 1) * s2' = sigmoid(q) * s2
                        nc.vector.scalar_tensor_tensor(
                            qq[:, sl], qq[:, sl], 1.0, w_ps[:], ALU.add, ALU.mult)
                    nc.scalar.dma_start(o_d[(t0 + u * QB) // QB], qq[:])
            pend_e = phase_e
        if pend_e is not None:
            pend_e()
    nc.compile()
    return nc


def _get_nc():
    if "nc" not in _CACHE:
        _CACHE["nc"] = _build()
    return _CACHE["nc"]


def _quadify(a):
    """[L, FREE] -> DMA-native [NTILES//4, P, 4*FREE] staged layout."""
    return np.ascontiguousarray(
        a.reshape(NTILES // 4, 4, P, FREE).transpose(0, 2, 1, 3)
    ).reshape(NTILES // 4, P, 4 * FREE)


def _stage2(a):
    """[L, RD] -> DVE-side [NLB, RD, LB] transposed layout."""
    # [L, RD] -> [RD, L] -> [RD, NLB, LB] -> [NLB, RD, LB]
    return np.ascontiguousarray(a.T.reshape(RD, NLB, LB).transpose(1, 0, 2))


def _run(queries, keys, values, key_lengths_add, trace=False, **kw):
    nc = _get_nc()
    U, IND128, SU, SEL, E7 = _constants()
    in_maps = []
    for c in range(NCORES):
        n = c // 2
        h0 = (c % 2) * HPC
        kk = keys[n, :, h0:h0 + HPC, :].reshape(L, HPC * E) \
            + key_lengths_add[n][:, None]
        qq = queries[n, :, h0:h0 + HPC, :].reshape(L, HPC * E)
        vv = values[n, :, h0:h0 + HPC, :].reshape(L, HPC * E) * 0.5
        kk16 = kk.astype(np.float16)
        qq16 = qq.astype(np.float16)
        vv16 = vv.astype(np.float16)
        in_maps.append({
            "queries": _quadify(qq16[:, :FREE]),
            "keys": _quadify(kk16[:, :FREE]),
            "values": _quadify(vv16[:, :FREE]),
            "queries2": _stage2(qq16[:, FREE:]),
            "keys2": _stage2(kk16[:, FREE:]),
            "values2": _stage2(vv16[:, FREE:]),
            "U": U, "IND128": IND128, "SU": SU, "SEL": SEL, "E7": E7,
        })
    res = run_bass_kernel_spmd(nc, in_maps, core_ids=list(range(NCORES)),
                               trace=trace, **kw)
    out = np.empty((N, L, H, E), dtype=np.float32)
    for c in range(NCORES):
        n = c // 2
        h0 = (c % 2) * HPC
        oc = res.results[c]["out"].reshape(NTILES // 4, P, 4, FREE)
        oc = oc.transpose(0, 2, 1, 3).reshape(L, FREE)
        o2 = res.results[c]["out2"]            # [NLB, RD, LB]
        o2 = o2.transpose(1, 0, 2).reshape(RD, L).T    # [L, RD]
        full = np.concatenate([oc, o2], axis=1)        # [L, 512]
        out[n, :, h0:h0 + HPC, :] = full.astype(np.float32).reshape(L, HPC, E)
    return out, res


def kernel(queries, keys, values, key_lengths_add):
    out, _ = _run(queries, keys, values, key_lengths_add)
    return out


if __name__ == "__main__":
    rng = np.random.default_rng(0)
    q = rng.standard_normal((N, L, H, E), dtype=np.float32)
    k = rng.standard_normal((N, L, H, E), dtype=np.float32)
    v = rng.standard_normal((N, L, H, E), dtype=np.float32)
    m = np.zeros((N, L), dtype=np.float32)
    o = kernel(q, k, v, m)
    print(o.shape, o.dtype, np.abs(o).mean())
